# revision 1
# baseline (speedup 1.0000x reference)
"""GCN layer (x@W1 -> relu -> @W2 -> weighted scatter-add over edges) on 8 TRN2 cores.

Strategy (two launches, 8-way SPMD):
  L1: row-shard x across cores; each core computes its shard of
      support = relu(x@W1+b1)@W2 via TensorE (fp32 accumulate), writes its
      [N/8, 128] bf16 table shard (64 features + 64-byte pad -> 256B rows,
      the dma_gather minimum row size). Host concatenates shards.
  L2: edges partitioned by destination shard (core = dst // (N/8)).
      Per core: dma_gather (GPSIMD SWDGE, 4 queues, 512-idx calls, deep
      buffering) fetches 256B table rows per edge; DVE scales messages by
      edge weight (batched tensor_tensor) and builds 0/1 one-hot matrices
      (batched is_equal against an iota tile); TensorE accumulates
      agg.T[64, dst_tile] += msg.T @ onehot in PSUM; bias b2 added at PSUM
      evacuation. Host transposes/concats shards.

All floating-point math happens on device; the host only shards, sorts edge
indices, and concatenates outputs.
"""

import sys

if "/opt/trn_rl_repo" not in sys.path:
    sys.path.insert(0, "/opt/trn_rl_repo")

import numpy as np

import concourse.bass as bass
import concourse.tile as tile
from concourse import library_config, mybir
from concourse.bass_utils import run_bass_kernel_spmd
from concourse.library_overlay import lower_extended_insts

F32 = mybir.dt.float32
BF16 = mybir.dt.bfloat16
I16 = mybir.dt.int16

NCORES = 8
NBLK = 4  # int16 gather-index blocks (N/NBLK must be < 32768)
R_TILES = 4  # dst tiles per gather range
GCAP = 4  # chunks (x128 idx) per dma_gather call
NQUEUES = 4  # SWDGE queues
DMA_SCRATCH = 16384  # SWDGE descriptor-ring carveout
ACT_FRAC = 0.0  # fraction of each group's message-scales run on ACT (rest DVE)
ROW = 128  # bf16 elements per table row (256B: 64 features + 64 pad)

MAX_WAITS = 1  # this walrus build rejects >1 semaphore wait per instruction


def _split_excess_waits(nc, max_waits=MAX_WAITS):
    """Move excess sem-waits onto injected same-engine NOPs placed before the
    over-subscribed instruction (same-engine program order keeps semantics)."""
    uid = 0
    for f in nc.m.functions:
        for bb in f.blocks:
            il = bb.instructions
            new_il = []
            for inst in il:
                si = inst.sync_info
                waits = list(si.on_wait) if si and si.on_wait else []
                if len(waits) > max_waits:
                    excess, keep = waits[:-max_waits], waits[-max_waits:]
                    for j in range(0, len(excess), max_waits):
                        grp = excess[j : j + max_waits]
                        nop = mybir.InstNoOp(name=f"I-waitsplit-{uid}", ins=[], outs=[])
                        uid += 1
                        nop.engine = inst.engine
                        nop.sync_info = mybir.SyncInfo(on_wait=grp, on_update=[])
                        nc.register_instruction(nop, overwrite=True)
                        new_il.append(nop)
                    si.on_wait = keep
                new_il.append(inst)
            il[:] = new_il


def _finalize(nc):
    lower_extended_insts(nc)
    _split_excess_waits(nc)


# ---------------------------------------------------------------- L1: matmuls


def _build_l1(n_nodes, nfeat, nhid, ncls):
    shard = n_nodes // NCORES
    nc = bass.Bass()
    xT = nc.dram_tensor("xT", [nfeat, shard], BF16, kind="ExternalInput")
    W1 = nc.dram_tensor("W1", [nfeat, nhid], BF16, kind="ExternalInput")
    b1 = nc.dram_tensor("b1", [nhid, 1], F32, kind="ExternalInput")
    W2 = nc.dram_tensor("W2", [nhid, ncls], F32, kind="ExternalInput")
    table = nc.dram_tensor("table", [shard, ROW], BF16, kind="ExternalOutput")

    kchunks = nfeat // 128
    assert nfeat % 128 == 0 and nhid == 128
    NCHW = 512  # node columns per h chunk
    nch = (shard + NCHW - 1) // NCHW
    ntiles = (shard + 127) // 128

    with tile.TileContext(nc) as tc:
        with (
            tc.tile_pool(name="const", bufs=1) as constp,
            tc.tile_pool(name="xbuf", bufs=3) as xbuf,
            tc.tile_pool(name="hbuf", bufs=1) as hbuf,
            tc.tile_pool(name="obuf", bufs=4) as obuf,
            tc.tile_pool(name="psh", bufs=4, space="PSUM") as psh,
            tc.tile_pool(name="pss", bufs=4, space="PSUM") as pss,
        ):
            w1s = constp.tile([128, kchunks, nhid], BF16)
            nc.sync.dma_start(
                out=w1s[:], in_=W1[:].rearrange("(k p) h -> p k h", p=128)
            )
            w2s = constp.tile([128, ncls], F32)
            nc.sync.dma_start(out=w2s[:], in_=W2[:])
            b1s = constp.tile([128, 1], F32)
            nc.sync.dma_start(out=b1s[:], in_=b1[:])

            hT = hbuf.tile([128, shard], F32)  # resident h.T (fp32)
            for j in range(nch):
                j0 = j * NCHW
                nsz = min(NCHW, shard - j0)
                xt = xbuf.tile([128, kchunks, NCHW], BF16, tag="xt")
                nc.sync.dma_start(
                    out=xt[:, :, :nsz],
                    in_=xT[:, j0 : j0 + nsz].rearrange("(k p) n -> p k n", p=128),
                )
                ph = psh.tile([128, NCHW], F32)
                for k in range(kchunks):
                    nc.tensor.matmul(
                        ph[:, :nsz],
                        w1s[:, k, :],
                        xt[:, k, :nsz],
                        start=(k == 0),
                        stop=(k == kchunks - 1),
                    )
                nc.scalar.activation(
                    hT[:, j0 : j0 + nsz],
                    ph[:, :nsz],
                    mybir.ActivationFunctionType.Relu,
                    bias=b1s[:],
                    scale=1.0,
                )
            for t in range(ntiles):
                t0 = t * 128
                msz = min(128, shard - t0)
                ps = pss.tile([128, ncls], F32)
                nc.tensor.matmul(
                    ps[:msz, :], hT[:, t0 : t0 + msz], w2s[:], start=True, stop=True
                )
                ob = obuf.tile([128, ROW], BF16, tag="ob")
                nc.vector.memset(ob[:, ncls:], 0.0)
                nc.vector.tensor_copy(ob[:msz, :ncls], ps[:msz, :])
                nc.sync.dma_start(out=table[t0 : t0 + msz, :], in_=ob[:msz, :])

    _finalize(nc)
    return nc


# ------------------------------------------------- edge schedule (host side)


def _edge_schedule(src, dst, ew, n_nodes, shard):
    """Partition edges by destination shard, sort by (dst tile, src block),
    build the SPMD-common gather/compute schedule (max counts over cores) and
    each core's index/weight streams laid into that skeleton.

    Returns (schedule, percore, dims).
    """
    blk = n_nodes // NBLK
    ntiles = (shard + 127) // 128
    core_of = dst // shard

    percore_edges = []
    cnt_all = np.zeros((NCORES, ntiles, NBLK), np.int64)
    for c in range(NCORES):
        m = core_of == c
        s = src[m]
        d = dst[m] - c * shard
        w = ew[m]
        tl = d // 128
        bl = s // blk
        order = np.lexsort((bl, tl))
        s, d, w, tl, bl = s[order], d[order], w[order], tl[order], bl[order]
        cnt = np.zeros((ntiles, NBLK), np.int64)
        np.add.at(cnt, (tl, bl), 1)
        cnt_all[c] = cnt
        run_off = np.zeros(ntiles * NBLK + 1, np.int64)
        np.cumsum(cnt.reshape(-1), out=run_off[1:])
        percore_edges.append((s, d, w, cnt, run_off))

    # SPMD skeleton: chunks per (tile, block) = ceil(max-over-cores / 128)
    nr_tb = (cnt_all.max(axis=0) + 127) // 128
    for t in range(ntiles):
        if nr_tb[t].sum() == 0:
            nr_tb[t, 0] = 1  # keep every tile non-empty

    nranges = (ntiles + R_TILES - 1) // R_TILES
    schedule = []
    icol_off = 0
    chunk_off = 0
    gmax = 1
    run_loc = {}  # (t, b) -> (icol, chunkcol, nchunks)
    for r in range(nranges):
        tlo, thi = r * R_TILES, min(ntiles, r * R_TILES + R_TILES)
        gathers = []
        for b in range(NBLK):
            nch_rb = int(nr_tb[tlo:thi, b].sum())
            if nch_rb == 0:
                continue
            ncalls = (nch_rb + GCAP - 1) // GCAP
            nch_pad = ncalls * GCAP
            off = 0
            for t in range(tlo, thi):
                if nr_tb[t, b]:
                    run_loc[(t, b)] = (
                        icol_off + off * 8,
                        chunk_off + off,
                        int(nr_tb[t, b]),
                    )
                    off += int(nr_tb[t, b])
            gathers.append(
                dict(
                    b=b,
                    icol=icol_off,
                    chunk0=chunk_off,
                    nchunks=nch_rb,
                    nch_pad=nch_pad,
                    ncalls=ncalls,
                )
            )
            gmax = max(gmax, nch_pad)
            icol_off += nch_pad * 8  # 128 idx per chunk = 8 cols of 16
            chunk_off += nch_rb
        tiles = []
        for t in range(tlo, thi):
            msz = min(128, shard - t * 128)
            runs = []
            for g in gathers:
                b = g["b"]
                if nr_tb[t, b]:
                    icol, chcol, nchk = run_loc[(t, b)]
                    runs.append((b, chcol - g["chunk0"], nchk, chcol))
            tiles.append(dict(t=t, msz=msz, runs=runs))
        schedule.append(dict(gathers=gathers, tiles=tiles))

    icols = max(icol_off, 16)
    tch = max(chunk_off, 1)

    percore = []
    for c in range(NCORES):
        s, d, w, cnt, run_off = percore_edges[c]
        idx_flat = np.zeros(tch * 128, np.int16)
        dst_flat = np.zeros(tch * 128, np.float32)
        w_flat = np.zeros(tch * 128, np.float32)
        for (t, b), (icol, chcol, nchk) in run_loc.items():
            n_real = int(cnt[t, b])
            if n_real == 0:
                continue
            i0 = int(run_off[t * NBLK + b])
            o0 = chcol * 128
            idx_flat[o0 : o0 + n_real] = (s[i0 : i0 + n_real] - b * blk).astype(
                np.int16
            )
            dst_flat[o0 : o0 + n_real] = (d[i0 : i0 + n_real] - t * 128).astype(
                np.float32
            )
            w_flat[o0 : o0 + n_real] = w[i0 : i0 + n_real]
        # lay real-chunk idx into the padded call skeleton; pad chunks gather
        # row 0 (finite, weight-0) so every call is a uniform full 512 idx
        idx_cols = np.zeros((icols // 8, 128), np.int16)
        for rng_ in schedule:
            for g in rng_["gathers"]:
                nch, c0 = g["nchunks"], g["chunk0"]
                block = idx_flat[c0 * 128 : (c0 + nch) * 128]
                base = g["icol"] // 8
                idx_cols[base : base + nch] = block.reshape(nch, 128)
        idx16 = np.tile(idx_cols.reshape(-1, 16).T, (8, 1))  # [128, icols]
        dstw = dst_flat.reshape(tch, 128).T.copy()
        wmat = w_flat.reshape(tch, 128).T.copy()
        percore.append(
            dict(
                idx=np.ascontiguousarray(idx16),
                dstw=np.ascontiguousarray(dstw),
                wmat=np.ascontiguousarray(wmat),
            )
        )

    fp = hash((nr_tb.tobytes(), shard, n_nodes))
    dims = dict(icols=icols, tch=tch, gmax=gmax, fingerprint=fp)
    return schedule, percore, dims


# ---------------------------------------------------------------- L2: edges


def _build_l2(n_nodes, ncls, shard, schedule, dims):
    blk = n_nodes // NBLK
    icols, tch, gmax = dims["icols"], dims["tch"], dims["gmax"]
    nc = bass.Bass(num_swdge_queues=NQUEUES, dynamic_dma_scratch_size=DMA_SCRATCH)
    table = nc.dram_tensor("table", [n_nodes, ROW], BF16, kind="ExternalInput")
    idxs = nc.dram_tensor("idxs", [128, icols], I16, kind="ExternalInput")
    dstw = nc.dram_tensor("dstw", [128, tch], BF16, kind="ExternalInput")
    wmat = nc.dram_tensor("wmat", [128, tch], BF16, kind="ExternalInput")
    b2t = nc.dram_tensor("b2t", [ncls, 1], F32, kind="ExternalInput")
    aggT = nc.dram_tensor("aggT", [ncls, shard], F32, kind="ExternalOutput")

    iota_np = np.tile(np.arange(128, dtype=np.float32), (128, 1))
    iota_t = nc.inline_tensor(iota_np, "iota")

    from contextlib import ExitStack

    with tile.TileContext(nc) as tc, ExitStack() as es:
        nidx_reg = es.enter_context(nc.gpsimd.register("nidx_reg"))
        with (
            tc.tile_pool(name="const", bufs=1) as constp,
            tc.tile_pool(name="idxp", bufs=2) as idxp,
            tc.tile_pool(name="gp", bufs=2) as gp,
            tc.tile_pool(name="ohp", bufs=5) as ohp,
            tc.tile_pool(name="evp", bufs=4) as evp,
            tc.tile_pool(name="psp", bufs=6, space="PSUM") as psp,
        ):
            nc.gpsimd.load_library(library_config.mlp)
            iota_f32 = constp.tile([128, 128], F32)
            nc.sync.dma_start(out=iota_f32[:], in_=iota_t[:])
            iota_s = constp.tile([128, 128], BF16)
            nc.vector.tensor_copy(iota_s[:], iota_f32[:])
            b2s = constp.tile([ncls, 1], F32)
            nc.sync.dma_start(out=b2s[:], in_=b2t[:])
            dstw_s = constp.tile([128, tch], BF16)
            nc.sync.dma_start(out=dstw_s[:], in_=dstw[:])
            wmat_s = constp.tile([128, tch], BF16)
            nc.sync.dma_start(out=wmat_s[:], in_=wmat[:])
            if ACT_FRAC > 0:
                wmat_f = constp.tile([128, tch], F32)
                nc.vector.tensor_copy(wmat_f[:], wmat_s[:])

            nc.gpsimd.reg_mov(nidx_reg, GCAP * 128)
            qn = 0
            gcount = 0
            for rng in schedule:
                gathers = rng["gathers"]
                gbufs = {}
                if gathers:
                    icol0 = gathers[0]["icol"]
                    icoln = gathers[-1]["icol"] + gathers[-1]["nch_pad"] * 8
                    ib = idxp.tile([128, icoln - icol0], I16, tag="idx")
                    nc.sync.dma_start(out=ib[:], in_=idxs[:, icol0:icoln])
                for g in gathers:
                    b = g["b"]
                    gb = gp.tile([128, g["nch_pad"], ROW], BF16, tag=f"g{b}")
                    gbufs[b] = g
                    g["tile"] = gb
                    nch = g["nchunks"]
                    for k in range(g["ncalls"]):
                        c_lo = k * GCAP
                        ic = g["icol"] - icol0 + c_lo * 8
                        nc.gpsimd.dma_gather(
                            gb[:, c_lo : c_lo + GCAP, :],
                            table[b * blk : (b + 1) * blk, :],
                            ib[:, ic : ic + GCAP * 8],
                            GCAP * 128,
                            nidx_reg,
                            ROW,
                            single_packet=True,
                            queue_num=qn,
                        )
                        qn = (qn + 1) % NQUEUES
                    # scale messages by edge weight, split between the idle
                    # ACT engine (per-chunk Copy-with-scale) and DVE (batched)
                    c0g = g["chunk0"]
                    nact = int(round(nch * ACT_FRAC))
                    for j in range(nact):
                        nc.scalar.activation(
                            gb[:, j, :ncls],
                            gb[:, j, :ncls],
                            mybir.ActivationFunctionType.Copy,
                            bias=0.0,
                            scale=wmat_f[:, c0g + j : c0g + j + 1],
                        )
                    if nact < nch:
                        nc.vector.tensor_tensor(
                            gb[:, nact:nch, :ncls],
                            gb[:, nact:nch, :ncls],
                            wmat_s[:, c0g + nact : c0g + nch]
                            .unsqueeze(2)
                            .to_broadcast((128, nch - nact, ncls)),
                            mybir.AluOpType.mult,
                        )
                    gcount += 1
                    # one batched 0/1 one-hot build per gather group (its
                    # chunk columns are contiguous across the range's tiles)
                    oh = ohp.tile([128, nch, 128], BF16, tag="oh")
                    g["oh"] = oh
                    nc.vector.tensor_tensor(
                        oh[:],
                        dstw_s[:, c0g : c0g + nch]
                        .unsqueeze(2)
                        .to_broadcast((128, nch, 128)),
                        iota_s[:].unsqueeze(1).to_broadcast((128, nch, 128)),
                        mybir.AluOpType.is_equal,
                    )
                for tt in rng["tiles"]:
                    t, msz, runs = tt["t"], tt["msz"], tt["runs"]
                    ps = psp.tile([ncls, 128], F32, tag="ps")
                    nchunks_t = sum(nr for (_, _, nr, _) in runs)
                    ci = 0
                    for b, c0, nr, chcol in runs:
                        g = gbufs[b]
                        gb = g["tile"]
                        oh = g["oh"]
                        for j in range(nr):
                            nc.tensor.matmul(
                                ps[:, :msz],
                                gb[:, c0 + j, :ncls],
                                oh[:, c0 + j, :msz],
                                start=(ci == 0),
                                stop=(ci == nchunks_t - 1),
                            )
                            ci += 1
                    ev = evp.tile([ncls, 128], F32, tag="ev")
                    nc.vector.tensor_scalar_add(ev[:, :msz], ps[:, :msz], b2s[:])
                    nc.sync.dma_start(
                        out=aggT[:, t * 128 : t * 128 + msz], in_=ev[:, :msz]
                    )

    _finalize(nc)
    return nc


# ------------------------------------------------------------------- driver

_CACHE = {}
LAST_TIMES = {}


def _timed_run(name, nc, in_maps, core_ids):
    import time as _time

    t0 = _time.time()
    res = run_bass_kernel_spmd(nc, in_maps, core_ids)
    LAST_TIMES[name] = _time.time() - t0
    return res


def kernel(x, W1, b1, W2, b2, edge_index, edge_weight):
    x = np.asarray(x, np.float32)
    W1 = np.asarray(W1, np.float32)
    b1 = np.asarray(b1, np.float32)
    W2 = np.asarray(W2, np.float32)
    b2 = np.asarray(b2, np.float32)
    edge_index = np.asarray(edge_index)
    edge_weight = np.asarray(edge_weight, np.float32)

    n_nodes, nfeat = x.shape
    ncls = W2.shape[1]
    shard = n_nodes // NCORES
    core_ids = list(range(NCORES))

    # ---- L1: support table ----
    key1 = ("l1", n_nodes, nfeat, W1.shape[1], ncls)
    if key1 not in _CACHE:
        _CACHE[key1] = _build_l1(n_nodes, nfeat, W1.shape[1], ncls)
    nc1 = _CACHE[key1]

    import ml_dtypes

    xT = np.ascontiguousarray(x.T).astype(ml_dtypes.bfloat16)
    W1b = W1.astype(ml_dtypes.bfloat16)
    in_maps1 = [
        {
            "xT": np.ascontiguousarray(xT[:, c * shard : (c + 1) * shard]),
            "W1": W1b,
            "b1": np.ascontiguousarray(b1.reshape(-1, 1)),
            "W2": W2,
        }
        for c in core_ids
    ]
    res1 = _timed_run("l1", nc1, in_maps1, core_ids)
    table = np.ascontiguousarray(
        np.concatenate([res1.results[c]["table"] for c in core_ids], axis=0)
    )

    # ---- host edge preprocessing ----
    src = edge_index[0].astype(np.int64)
    dst = edge_index[1].astype(np.int64)
    ekey = ("sched", n_nodes, shard, edge_index.shape[1])
    if ekey in _CACHE and _CACHE[ekey][0] is not None:
        fph, schedule, percore, dims = _CACHE[ekey]
        if fph != hash(edge_index.tobytes()):
            schedule = None
    else:
        schedule = None
    if schedule is None:
        schedule, percore, dims = _edge_schedule(
            src, dst, edge_weight, n_nodes, shard
        )
        _CACHE[ekey] = (hash(edge_index.tobytes()), schedule, percore, dims)

    key2 = ("l2", n_nodes, ncls, shard, dims["fingerprint"])
    if key2 not in _CACHE:
        _CACHE[key2] = _build_l2(n_nodes, ncls, shard, schedule, dims)
    nc2 = _CACHE[key2]

    import ml_dtypes

    b2c = np.ascontiguousarray(b2.reshape(-1, 1))
    in_maps2 = [
        {
            "table": table,
            "idxs": percore[c]["idx"],
            "dstw": percore[c]["dstw"].astype(ml_dtypes.bfloat16),
            "wmat": percore[c]["wmat"].astype(ml_dtypes.bfloat16),
            "b2t": b2c,
        }
        for c in core_ids
    ]
    res2 = _timed_run("l2", nc2, in_maps2, core_ids)
    out = np.concatenate(
        [np.ascontiguousarray(res2.results[c]["aggT"].T.astype(np.float32)) for c in core_ids],
        axis=0,
    )
    return out



# revision 16
# speedup vs baseline: 1.6125x; 1.6125x over previous
"""GCN layer (x@W1 -> relu -> @W2 -> weighted scatter-add over edges) on 8 TRN2 cores.

Strategy (two launches, 8-way SPMD):
  L1: row-shard x across cores; each core computes its shard of
      support = relu(x@W1+b1)@W2 via TensorE (fp32 accumulate), writes its
      [N/8, 128] bf16 table shard (64 features + 64B pad -> 256B rows).
      Host concatenates shards.
  L2: edges partitioned by destination shard (core = dst // (N/8)).
      Per core: one dma_gather per (512-dst range, src block) fetches the
      first 128B (64 bf16 features) of each edge's 256B table row (small
      descriptors cost half of 256B ones); DVE builds w-valued one-hots in a
      swapped [128, 64, nch] layout (keeps the 2x perf mode); TensorE
      accumulates per-32-dst-tile psum[64f, 32d] with the one-hot as the
      narrow moving operand; ACT evacuates psum + bias into a resident
      aggT buffer written out with one large DMA. Host transposes/trims.

Edge streams are max-aligned across cores per (tile, block) so the chunk ->
tile map is SPMD-common; chunks may span two adjacent 32-dst tiles and are
then matmul'd into both (one-hot cols [0:32] / [32:64]).
"""

import sys

if "/opt/trn_rl_repo" not in sys.path:
    sys.path.insert(0, "/opt/trn_rl_repo")

import numpy as np

import concourse.bass as bass
import concourse.tile as tile
from concourse import library_config, mybir
from concourse.bass_utils import run_bass_kernel_spmd
from concourse.library_overlay import lower_extended_insts

F32 = mybir.dt.float32
BF16 = mybir.dt.bfloat16
I16 = mybir.dt.int16

NCORES = 8
NBLK = 4  # int16 gather-index blocks (N/NBLK must be < 32768)
R_DST = 512  # dsts per gather range
T_W = 32  # dsts per psum tile
OHW = 64  # one-hot width (covers a chunk spanning two adjacent tiles)
NQUEUES = 4  # SWDGE queues
DMA_SCRATCH = 16384  # SWDGE descriptor-ring carveout
ROW = 128  # bf16 elements per table row (256B)
GELEM = 64  # gathered bf16 elements per edge (128B descriptors)

MAX_WAITS = 1  # this walrus build rejects >1 semaphore wait per instruction


def _split_excess_waits(nc, max_waits=MAX_WAITS):
    """Move excess sem-waits onto injected same-engine NOPs placed before the
    over-subscribed instruction (same-engine program order keeps semantics)."""
    uid = 0
    for f in nc.m.functions:
        for bb in f.blocks:
            il = bb.instructions
            new_il = []
            for inst in il:
                si = inst.sync_info
                waits = list(si.on_wait) if si and si.on_wait else []
                if len(waits) > max_waits:
                    excess, keep = waits[:-max_waits], waits[-max_waits:]
                    for j in range(0, len(excess), max_waits):
                        grp = excess[j : j + max_waits]
                        nop = mybir.InstNoOp(name=f"I-waitsplit-{uid}", ins=[], outs=[])
                        uid += 1
                        nop.engine = inst.engine
                        nop.sync_info = mybir.SyncInfo(on_wait=grp, on_update=[])
                        nc.register_instruction(nop, overwrite=True)
                        new_il.append(nop)
                    si.on_wait = keep
                new_il.append(inst)
            il[:] = new_il


def _finalize(nc):
    lower_extended_insts(nc)
    _split_excess_waits(nc)


def _dma_gather_raw(
    engine,
    out_ap,
    in_ap,
    idxs_ap,
    num_idxs,
    num_idxs_reg,
    elem_size,
    elem_step,
    queue_num,
    single_packet=True,
):
    """nc.gpsimd.dma_gather minus the 256B-multiple elem_size assert (the
    restriction is transpose-only in both the ucode and the decode path).
    elem_size / elem_step in elements of in_ap.dtype; stride must still be a
    256B multiple."""
    from concourse import ap_utils
    from concourse.bass import MemorySpace
    from concourse._compat import exact_div, round_up_to_multiple

    self = engine
    self._assert_queue_num(queue_num)
    assert idxs_ap.dtype == mybir.dt.int16
    assert in_ap.space == MemorySpace.DRAM
    assert idxs_ap.space == MemorySpace.SBUF
    assert out_ap.space == MemorySpace.SBUF
    assert in_ap.dtype == out_ap.dtype
    assert ap_utils.ap_is_contiguous(out_ap.ap[1:])
    assert ap_utils.ap_is_contiguous(idxs_ap.ap[1:])
    assert in_ap.ap[-1][1] == out_ap.ap[-1][1] == elem_size

    assert out_ap.ap[0][1] * out_ap.ap[1][1] == round_up_to_multiple(num_idxs, 128)
    assert in_ap.ap[0][0] == elem_step
    stride_bytes = elem_step * mybir.dt.size(in_ap.dtype)
    stride_bytes_256 = exact_div(stride_bytes, 256)
    assert stride_bytes_256 < 256

    _in_ap = self.lower_ap_dma(in_ap, for_custom_bir_dma=True)
    _idxs_ap = self.lower_ap(idxs_ap)
    _out_ap = self.lower_ap(out_ap)
    return self.add_instruction(
        mybir.InstDMAGatherAnt(
            name=self.bass.get_next_instruction_name(),
            ins=[
                *_in_ap,
                _idxs_ap,
                self.lower_val_access(self.to_reg(num_idxs_reg)),
            ],
            outs=[_out_ap],
            transpose=False,
            num_idxs=num_idxs,
            elem_size=elem_size,
            stride_bytes_256=stride_bytes_256,
            gen_mode=0,
            single_packet=single_packet,
            queue_num=queue_num,
            sbuf_tokens_per_rank=0,
            sbuf_free_dim_per_rank=0,
            sbuf_free_dim_pad_per_rank=0,
            sbuf_byte_offset=0,
        )
    )


# ---------------------------------------------------------------- L1: matmuls


def _build_l1(n_nodes, nfeat, nhid, ncls):
    shard = n_nodes // NCORES
    nc = bass.Bass()
    xT = nc.dram_tensor("xT", [nfeat, shard], BF16, kind="ExternalInput")
    W1 = nc.dram_tensor("W1", [nfeat, nhid], BF16, kind="ExternalInput")
    b1 = nc.dram_tensor("b1", [nhid, 1], F32, kind="ExternalInput")
    W2 = nc.dram_tensor("W2", [nhid, ncls], F32, kind="ExternalInput")
    table = nc.dram_tensor("table", [shard, ROW], BF16, kind="ExternalOutput")

    kchunks = nfeat // 128
    assert nfeat % 128 == 0 and nhid == 128
    NCHW = 512  # node columns per h chunk
    nch = (shard + NCHW - 1) // NCHW
    ntiles = (shard + 127) // 128
    TBATCH = 16  # table tiles per output DMA

    with tile.TileContext(nc) as tc:
        with (
            tc.tile_pool(name="const", bufs=1) as constp,
            tc.tile_pool(name="xbuf", bufs=3) as xbuf,
            tc.tile_pool(name="hbuf", bufs=1) as hbuf,
            tc.tile_pool(name="obuf", bufs=2) as obuf,
            tc.tile_pool(name="psh", bufs=4, space="PSUM") as psh,
            tc.tile_pool(name="pss", bufs=4, space="PSUM") as pss,
        ):
            w1s = constp.tile([128, kchunks, nhid], BF16)
            nc.sync.dma_start(
                out=w1s[:], in_=W1[:].rearrange("(k p) h -> p k h", p=128)
            )
            w2s = constp.tile([128, ncls], F32)
            nc.sync.dma_start(out=w2s[:], in_=W2[:])
            b1s = constp.tile([128, 1], F32)
            nc.sync.dma_start(out=b1s[:], in_=b1[:])

            hT = hbuf.tile([128, shard], F32)  # resident h.T (fp32)
            for j in range(nch):
                j0 = j * NCHW
                nsz = min(NCHW, shard - j0)
                xt = xbuf.tile([128, kchunks, NCHW], BF16, tag="xt")
                nc.sync.dma_start(
                    out=xt[:, :, :nsz],
                    in_=xT[:, j0 : j0 + nsz].rearrange("(k p) n -> p k n", p=128),
                )
                ph = psh.tile([128, NCHW], F32)
                for k in range(kchunks):
                    nc.tensor.matmul(
                        ph[:, :nsz],
                        w1s[:, k, :],
                        xt[:, k, :nsz],
                        start=(k == 0),
                        stop=(k == kchunks - 1),
                    )
                nc.scalar.activation(
                    hT[:, j0 : j0 + nsz],
                    ph[:, :nsz],
                    mybir.ActivationFunctionType.Relu,
                    bias=b1s[:],
                    scale=1.0,
                )
            for t0 in range(0, ntiles, TBATCH):
                tn = min(TBATCH, ntiles - t0)
                ob = obuf.tile([128, TBATCH, ROW], BF16, tag="ob")
                nc.vector.memset(ob[:, :tn, ncls:], 0.0)
                for tt in range(tn):
                    t = t0 + tt
                    msz = min(128, shard - t * 128)
                    ps = pss.tile([128, ncls], F32)
                    nc.tensor.matmul(
                        ps[:msz, :],
                        hT[:, t * 128 : t * 128 + msz],
                        w2s[:],
                        start=True,
                        stop=True,
                    )
                    nc.vector.tensor_copy(ob[:msz, tt, :ncls], ps[:msz, :])
                rows = min(TBATCH * 128, shard - t0 * 128)
                full = rows // 128 * 128
                if full:
                    nc.sync.dma_start(
                        out=table[t0 * 128 : t0 * 128 + full, :].rearrange(
                            "(k p) r -> p k r", p=128
                        ),
                        in_=ob[:, : full // 128, :],
                    )
                if rows > full:
                    nc.sync.dma_start(
                        out=table[t0 * 128 + full : t0 * 128 + rows, :],
                        in_=ob[: rows - full, full // 128, :],
                    )

    _finalize(nc)
    return nc


# ------------------------------------------------- edge schedule (host side)


def _edge_schedule(src, dst, ew, n_nodes, shard):
    """Partition edges by destination shard; per (512-dst range, src block)
    sort by dst and max-align per (32-dst tile, block) across cores so the
    chunk->tile map is SPMD-common.  Build the gather/matmul schedule and each
    core's idx/dstw/w streams laid into that skeleton.

    Returns (schedule, percore, dims).
    """
    blk = n_nodes // NBLK
    ntile = (shard + T_W - 1) // T_W  # 32-dst tiles per core
    nrange = (shard + R_DST - 1) // R_DST
    tpr = R_DST // T_W  # tiles per range (last range partial)

    # per-core edge arrays grouped by (range, block, tile), dst-sorted
    percore_raw = []
    cnt_all = np.zeros((NCORES, ntile, NBLK), np.int64)
    for c in range(NCORES):
        m = (dst >= c * shard) & (dst < (c + 1) * shard)
        s = src[m]
        d = dst[m] - c * shard
        w = ew[m]
        tl = d // T_W
        b = s // blk
        order = np.lexsort((d, b, tl))  # tile-major, then block, then dst
        s, d, w, tl, b = s[order], d[order], w[order], tl[order], b[order]
        cnt = np.zeros((ntile, NBLK), np.int64)
        np.add.at(cnt, (tl, b), 1)
        cnt_all[c] = cnt
        off = np.zeros(ntile * NBLK + 1, np.int64)
        np.cumsum(cnt.reshape(-1), out=off[1:])
        percore_raw.append((s, d, w, off))

    # SPMD-common per-(tile, block) counts: max over cores, min 129 so a
    # 128-edge chunk can span at most two adjacent tiles
    maxcnt = cnt_all.max(axis=0)
    maxcnt = np.maximum(maxcnt, 129)

    # Uniform gather-call size: every (range, block) group spans `cap` chunks
    # (one reg_mov, identical static num_idxs on every call — no scheduler
    # hazard on the GPSIMD count register).  Trailing pad chunks are gathered
    # (row 0) but have no tile uses.
    cap = 1
    for r in range(nrange):
        t_lo = r * tpr
        t_hi = min(ntile, t_lo + tpr)
        for b in range(NBLK):
            total = int(maxcnt[t_lo:t_hi, b].sum())
            cap = max(cap, (total + 127) // 128)

    schedule = []
    icol_off = 0
    chunk_off = 0
    for r in range(nrange):
        t_lo = r * tpr
        t_hi = min(ntile, t_lo + tpr)
        gathers = []
        tiles = [dict(t=t, uses=[]) for t in range(t_lo, t_hi)]
        for b in range(NBLK):
            # common cumulative edge positions of each tile within the stream
            cum = np.zeros(t_hi - t_lo + 1, np.int64)
            np.cumsum(maxcnt[t_lo:t_hi, b], out=cum[1:])
            total = int(cum[-1])
            nch = (total + 127) // 128
            assert nch <= cap
            # primary tile of chunk k = tile containing stream position 128k
            prim = np.searchsorted(cum, np.arange(nch) * 128, side="right") - 1
            for ti, t in enumerate(range(t_lo, t_hi)):
                c_first = int(cum[ti]) // 128
                c_last = (int(cum[ti + 1]) - 1) // 128
                for ck in range(c_first, c_last + 1):
                    span = ti - int(prim[ck])
                    assert 0 <= span <= 1
                    tiles[ti]["uses"].append((b, ck, T_W * span))
            gathers.append(
                dict(b=b, icol=icol_off, chunk0=chunk_off, nch=nch, cum=cum, prim=prim)
            )
            icol_off += cap * 8
            chunk_off += cap
        schedule.append(dict(r=r, gathers=gathers, tiles=tiles))

    icols = max(icol_off, 16)
    tch = max(chunk_off, 1)

    percore = []
    for c in range(NCORES):
        s, d, w, off = percore_raw[c]
        idx_flat = np.zeros(tch * 128, np.int16)
        dstw_flat = np.zeros(tch * 128, np.float32)
        w_flat = np.zeros(tch * 128, np.float32)
        for rng_ in schedule:
            r = rng_["r"]
            t_lo = r * tpr
            for g in rng_["gathers"]:
                b, cum, prim, c0 = g["b"], g["cum"], g["prim"], g["chunk0"]
                base = c0 * 128
                for ti in range(len(cum) - 1):
                    t = t_lo + ti
                    n_real = int(cnt_all[c, t, b])
                    i0 = int(off[t * NBLK + b])
                    p0 = base + int(cum[ti])
                    # real edges
                    idx_flat[p0 : p0 + n_real] = (
                        s[i0 : i0 + n_real] - b * blk
                    ).astype(np.int16)
                    w_flat[p0 : p0 + n_real] = w[i0 : i0 + n_real]
                    # dstw relative to the chunk's primary tile
                    pos = np.arange(p0, p0 + n_real)
                    ck = pos // 128 - c0
                    dstw_flat[p0 : p0 + n_real] = (
                        d[i0 : i0 + n_real] - (t_lo + prim[ck]) * T_W
                    ).astype(np.float32)
                    # pads (idx 0, w 0, dstw 0) are already zero-filled
        idx_cols = idx_flat.reshape(tch, 128)
        idx16 = np.tile(
            idx_cols.reshape(-1, 16).T, (8, 1)
        )  # [128, icols] wrapped+replicated
        dstw = dstw_flat.reshape(tch, 128).T.copy()
        wmat = w_flat.reshape(tch, 128).T.copy()
        percore.append(
            dict(
                idx=np.ascontiguousarray(idx16),
                dstw=np.ascontiguousarray(dstw),
                wmat=np.ascontiguousarray(wmat),
            )
        )

    fp = hash((maxcnt.tobytes(), shard, n_nodes))
    dims = dict(icols=icols, tch=tch, cap=cap, ntile=ntile, fingerprint=fp)
    return schedule, percore, dims


# ---------------------------------------------------------------- L2: edges


def _build_l2(
    n_nodes,
    ncls,
    shard,
    schedule,
    dims,
    gelem=GELEM,
    swap_oh=True,
    do_gather=True,
    do_mm=True,
    subcap=None,
    scratch=DMA_SCRATCH,
    single_packet=False,
):
    blk = n_nodes // NBLK
    icols, tch, cap, ntile = dims["icols"], dims["tch"], dims["cap"], dims["ntile"]
    if subcap is None:
        subcap = cap
    assert cap % subcap == 0
    ncols = ntile * T_W  # aggT columns (>= shard, host trims)
    nc = bass.Bass(num_swdge_queues=NQUEUES, dynamic_dma_scratch_size=scratch)
    table = nc.dram_tensor("table", [n_nodes, ROW], BF16, kind="ExternalInput")
    idxs = nc.dram_tensor("idxs", [128, icols], I16, kind="ExternalInput")
    dstw = nc.dram_tensor("dstw", [128, tch], BF16, kind="ExternalInput")
    wmat = nc.dram_tensor("wmat", [128, tch], BF16, kind="ExternalInput")
    b2t = nc.dram_tensor("b2t", [ncls, 1], F32, kind="ExternalInput")
    aggT = nc.dram_tensor("aggT", [ncls, ncols], F32, kind="ExternalOutput")

    # iota values 0..OHW-1 along dim1, replicated along dim2 (chunk cols)
    iota_np = np.tile(
        np.arange(OHW, dtype=np.float32)[None, :, None], (128, 1, cap)
    )

    from contextlib import ExitStack

    with tile.TileContext(nc) as tc, ExitStack() as es:
        nidx_reg = es.enter_context(nc.gpsimd.register("nidx_reg"))
        with (
            tc.tile_pool(name="const", bufs=1) as constp,
            tc.tile_pool(name="idxp", bufs=2) as idxp,
            tc.tile_pool(name="gp", bufs=2) as gp,
            tc.tile_pool(name="ohp", bufs=2) as ohp,
            tc.tile_pool(name="psp", bufs=8, space="PSUM") as psp,
        ):
            nc.gpsimd.load_library(library_config.mlp)
            iota_t = nc.inline_tensor(
                np.ascontiguousarray(iota_np.astype(np.float32)[:, :, 0]), "iota"
            )
            iota_f32 = constp.tile([128, OHW], F32)
            nc.sync.dma_start(out=iota_f32[:], in_=iota_t[:])
            iota_mat = constp.tile([128, OHW, cap], BF16)
            nc.vector.tensor_copy(
                iota_mat[:], iota_f32[:].unsqueeze(2).to_broadcast((128, OHW, cap))
            )
            b2s = constp.tile([ncls, 1], F32)
            nc.sync.dma_start(out=b2s[:], in_=b2t[:])
            dstw_s = constp.tile([128, tch], BF16)
            nc.sync.dma_start(out=dstw_s[:], in_=dstw[:])
            wmat_s = constp.tile([128, tch], BF16)
            nc.sync.dma_start(out=wmat_s[:], in_=wmat[:])
            aggb = constp.tile([ncls, ncols], F32)

            nc.gpsimd.reg_mov(nidx_reg, subcap * 128)
            qn = 0
            for rng in schedule:
                gathers = rng["gathers"]
                icol0 = gathers[0]["icol"]
                icoln = gathers[-1]["icol"] + cap * 8
                ib = idxp.tile([128, icoln - icol0], I16, tag="idx")
                nc.sync.dma_start(out=ib[:], in_=idxs[:, icol0:icoln])
                gbs = {}
                ohs = {}
                for g in gathers:
                    b = g["b"]
                    nch = g["nch"]
                    gb = gp.tile([128, cap, gelem], BF16, tag=f"g{b}")
                    gbs[b] = (gb, g["chunk0"])
                    if do_gather:
                        ic0 = g["icol"] - icol0
                        for k in range(0, cap, subcap):
                            _dma_gather_raw(
                                nc.gpsimd,
                                gb[:, k : k + subcap, :],
                                table[b * blk : (b + 1) * blk, :gelem],
                                ib[:, ic0 + k * 8 : ic0 + (k + subcap) * 8],
                                subcap * 128,
                                nidx_reg,
                                gelem,
                                ROW,
                                queue_num=qn,
                                single_packet=single_packet,
                            )
                            qn = (qn + 1) % NQUEUES
                    else:
                        nc.vector.memset(gb[:], 0.0)
                    if not do_mm:
                        continue
                    c0 = g["chunk0"]
                    if swap_oh:
                        oh = ohp.tile([128, OHW, cap], BF16, tag=f"oh{b}")
                        ohs[b] = oh
                        # w-valued one-hot, swapped layout for DVE 2x mode
                        nc.vector.tensor_tensor(
                            oh[:, :, :nch],
                            dstw_s[:, c0 : c0 + nch]
                            .unsqueeze(1)
                            .to_broadcast((128, OHW, nch)),
                            iota_mat[:, :, :nch],
                            mybir.AluOpType.is_equal,
                        )
                        nc.vector.tensor_tensor(
                            oh[:, :, :nch],
                            oh[:, :, :nch],
                            wmat_s[:, c0 : c0 + nch]
                            .unsqueeze(1)
                            .to_broadcast((128, OHW, nch)),
                            mybir.AluOpType.mult,
                        )
                    else:
                        oh = ohp.tile([128, cap, OHW], BF16, tag=f"oh{b}")
                        ohs[b] = oh
                        nc.vector.tensor_tensor(
                            oh[:, :nch, :],
                            dstw_s[:, c0 : c0 + nch]
                            .unsqueeze(2)
                            .to_broadcast((128, nch, OHW)),
                            iota_mat[:, :, 0]
                            .unsqueeze(1)
                            .to_broadcast((128, nch, OHW)),
                            mybir.AluOpType.is_equal,
                        )
                        nc.vector.tensor_tensor(
                            oh[:, :nch, :],
                            oh[:, :nch, :],
                            wmat_s[:, c0 : c0 + nch]
                            .unsqueeze(2)
                            .to_broadcast((128, nch, OHW)),
                            mybir.AluOpType.mult,
                        )
                if not do_mm:
                    continue
                for tt in rng["tiles"]:
                    t, uses = tt["t"], tt["uses"]
                    ps = psp.tile([ncls, T_W], F32, tag="ps")
                    nuse = len(uses)
                    for ui, (b, ck, ohoff) in enumerate(uses):
                        gb, _ = gbs[b]
                        oh = ohs[b]
                        rhs = (
                            oh[:, ohoff : ohoff + T_W, ck]
                            if swap_oh
                            else oh[:, ck, ohoff : ohoff + T_W]
                        )
                        nc.tensor.matmul(
                            ps[:],
                            gb[:, ck, :],
                            rhs,
                            start=(ui == 0),
                            stop=(ui == nuse - 1),
                        )
                    # evacuate on ACT with the bias folded in
                    nc.scalar.activation(
                        aggb[:, t * T_W : (t + 1) * T_W],
                        ps[:],
                        mybir.ActivationFunctionType.Identity,
                        bias=b2s[:],
                        scale=1.0,
                    )
            if not do_mm:
                nc.vector.memset(aggb[:], 0.0)
            nc.sync.dma_start(out=aggT[:], in_=aggb[:])

    _finalize(nc)
    return nc


# ------------------------------------------------------------------- driver

_CACHE = {}
LAST_TIMES = {}


def _timed_run(name, nc, in_maps, core_ids):
    import time as _time

    t0 = _time.time()
    res = run_bass_kernel_spmd(nc, in_maps, core_ids)
    LAST_TIMES[name] = _time.time() - t0
    return res


def kernel(x, W1, b1, W2, b2, edge_index, edge_weight):
    x = np.asarray(x, np.float32)
    W1 = np.asarray(W1, np.float32)
    b1 = np.asarray(b1, np.float32)
    W2 = np.asarray(W2, np.float32)
    b2 = np.asarray(b2, np.float32)
    edge_index = np.asarray(edge_index)
    edge_weight = np.asarray(edge_weight, np.float32)

    n_nodes, nfeat = x.shape
    ncls = W2.shape[1]
    shard = n_nodes // NCORES
    core_ids = list(range(NCORES))

    # ---- L1: support table ----
    key1 = ("l1", n_nodes, nfeat, W1.shape[1], ncls)
    if key1 not in _CACHE:
        _CACHE[key1] = _build_l1(n_nodes, nfeat, W1.shape[1], ncls)
    nc1 = _CACHE[key1]

    import ml_dtypes

    xT = np.ascontiguousarray(x.T).astype(ml_dtypes.bfloat16)
    W1b = W1.astype(ml_dtypes.bfloat16)
    in_maps1 = [
        {
            "xT": np.ascontiguousarray(xT[:, c * shard : (c + 1) * shard]),
            "W1": W1b,
            "b1": np.ascontiguousarray(b1.reshape(-1, 1)),
            "W2": W2,
        }
        for c in core_ids
    ]
    res1 = _timed_run("l1", nc1, in_maps1, core_ids)
    table = np.ascontiguousarray(
        np.concatenate([res1.results[c]["table"] for c in core_ids], axis=0)
    )

    # ---- host edge preprocessing ----
    src = edge_index[0].astype(np.int64)
    dst = edge_index[1].astype(np.int64)
    ekey = ("sched", n_nodes, shard, edge_index.shape[1])
    if ekey in _CACHE and _CACHE[ekey][0] == hash(edge_index.tobytes()):
        _, schedule, percore, dims = _CACHE[ekey]
    else:
        schedule, percore, dims = _edge_schedule(
            src, dst, edge_weight, n_nodes, shard
        )
        _CACHE[ekey] = (hash(edge_index.tobytes()), schedule, percore, dims)

    key2 = ("l2", n_nodes, ncls, shard, dims["fingerprint"])
    if key2 not in _CACHE:
        _CACHE[key2] = _build_l2(n_nodes, ncls, shard, schedule, dims)
    nc2 = _CACHE[key2]

    b2c = np.ascontiguousarray(b2.reshape(-1, 1))
    in_maps2 = [
        {
            "table": table,
            "idxs": percore[c]["idx"],
            "dstw": percore[c]["dstw"].astype(ml_dtypes.bfloat16),
            "wmat": percore[c]["wmat"].astype(ml_dtypes.bfloat16),
            "b2t": b2c,
        }
        for c in core_ids
    ]
    res2 = _timed_run("l2", nc2, in_maps2, core_ids)
    out = np.concatenate(
        [
            np.ascontiguousarray(
                res2.results[c]["aggT"].T[:shard].astype(np.float32)
            )
            for c in core_ids
        ],
        axis=0,
    )
    return out
